# revision 1
# baseline (speedup 1.0000x reference)
"""KMeans assignment (vq_codebook) Trainium2 kernel.

argmin_k ||x_b - c_k||^2 for X[65536,1024], C[1024,1024], 8 NeuronCores,
data-parallel over the batch (8192 rows/core), centroids replicated.

Math: argmin_k d2 = argmax_k (X@C^T - ||c||^2/2); row term ||x||^2 dropped.
The cross term is computed to ~fp32 accuracy with 3 float32r matmuls via an
exact hi/lo mantissa split (fp22 truncation makes each product exact):
  X = Xh + Xl, C = Ch + Cl (hi = top 11 mantissa bits)
  X@C^T ~= Xh@Ch^T + Xh@Cl^T + Xl@Ch^T   (dropped Xl@Cl^T ~ 2^-24 rel)
The ||c||^2/2 bias is computed on device, broadcast to all partitions, and
subtracted on the Vector engine; argmax uses the DVE max/max_index ops.
"""
import numpy as np
import concourse.bacc as bacc
import concourse.mybir as mybir
from concourse.tile import TileContext
from concourse.bass_utils import run_bass_kernel_spmd

B, F, K = 65536, 1024, 1024
NCORES = 8
BL = B // NCORES          # rows per core
P = 128
FCH = F // P              # 8 contraction chunks
NH = 512                  # psum half (max fp32 moving operand / bank)
BBLK = 1024               # rows per X DMA block (2KB lines: full DMA bandwidth)
NBLK = BL // BBLK
TPB = BBLK // P           # b-tiles per block
DT = mybir.dt.bfloat16

_NC_CACHE = {}


def _build(bl):
    nblk = bl // BBLK
    nb = bl // P
    nc = bacc.Bacc("TRN2", target_bir_lowering=False)
    xh = nc.dram_tensor("xh", [F, bl], DT, kind="ExternalInput")
    xl = nc.dram_tensor("xl", [F, bl], DT, kind="ExternalInput")
    ch = nc.dram_tensor("ch", [F, K], DT, kind="ExternalInput")
    cl = nc.dram_tensor("cl", [F, K], DT, kind="ExternalInput")
    cc = nc.dram_tensor("cc", [K, F], mybir.dt.float32, kind="ExternalInput")
    out = nc.dram_tensor("out", [nb, P, 1], mybir.dt.uint32, kind="ExternalOutput")
    c2lin = nc.dram_tensor("c2lin", [K], mybir.dt.float32, kind="Internal")

    xh_r = xh.rearrange("(fo p) b -> p fo b", p=P)
    xl_r = xl.rearrange("(fo p) b -> p fo b", p=P)

    with TileContext(nc) as tc:
        with (
            tc.tile_pool(name="cres", bufs=1) as cres,
            tc.tile_pool(name="xp", bufs=2) as xp,
            tc.tile_pool(name="work", bufs=3) as work,
            tc.tile_pool(name="psp", bufs=4, space="PSUM") as psp,
        ):
            # resident transposed centroid tiles (hi/lo split); one tile per
            # f-chunk so the first matmul only waits on chunk 0's DMA.
            # Issue order: C chunks + block-0 X chunks first (PE-critical),
            # cc + the c2 chain after (only needed by the first DVE sub,
            # which PSUM bufs=4 pushes ~40us out).
            def load_blk_chunk(blk, f):
                b0 = blk * BBLK
                t_h = xp.tile([P, BBLK], DT, tag=f"xh{f}")
                t_l = xp.tile([P, BBLK], DT, tag=f"xl{f}")
                nc.sync.dma_start(t_h, xh[f * P:(f + 1) * P, b0:b0 + BBLK])
                nc.sync.dma_start(t_l, xl[f * P:(f + 1) * P, b0:b0 + BBLK])
                return t_h, t_l

            def load_blk(blk):
                hs, ls = [], []
                for f in range(FCH):
                    t_h, t_l = load_blk_chunk(blk, f)
                    hs.append(t_h)
                    ls.append(t_l)
                return hs, ls

            # C chunks first (PE-critical, resident for the whole kernel),
            # then block-0's X chunks. Per-chunk tiles keep the first
            # matmuls gated only on the chunks they actually read.
            ch_sb = []
            cl_sb = []
            for f in range(FCH):
                t_h = cres.tile([P, K], DT, tag=f"ch{f}")
                t_l = cres.tile([P, K], DT, tag=f"cl{f}")
                nc.sync.dma_start(t_h, ch[f * P:(f + 1) * P, :])
                nc.sync.dma_start(t_l, cl[f * P:(f + 1) * P, :])
                ch_sb.append(t_h)
                cl_sb.append(t_l)

            blk0_tiles = load_blk(0)

            # c2/2 on device from row-major centroids (segmented reduce for
            # better fp32 accuracy), then scatter->broadcast via DRAM.
            c2pm = cres.tile([P, FCH], mybir.dt.float32)
            for j in range(FCH):
                cc_sb = work.tile([P, F], mybir.dt.float32, tag="ccsb")
                nc.sync.dma_start(cc_sb, cc[j * P:(j + 1) * P, :])
                csq = work.tile([P, F], mybir.dt.float32, tag="csq")
                nc.vector.tensor_mul(csq, cc_sb, cc_sb)
                seg = work.tile([P, 16], mybir.dt.float32, tag="seg")
                nc.vector.tensor_reduce(
                    seg, csq.rearrange("p (s t) -> p s t", t=64),
                    axis=mybir.AxisListType.X, op=mybir.AluOpType.add)
                nc.vector.tensor_reduce(
                    c2pm[:, j:j + 1], seg,
                    axis=mybir.AxisListType.X, op=mybir.AluOpType.add)
            nc.vector.tensor_scalar_mul(c2pm, c2pm, 0.5)
            nc.sync.dma_start(c2lin.rearrange("(j p) -> p j", p=P), c2pm)
            c2b = cres.tile([P, K], mybir.dt.float32)
            nc.sync.dma_start(c2b, c2lin[None, :].to_broadcast([P, K]))

            for blk in range(nblk):
                xh_t, xl_t = blk0_tiles if blk == 0 else load_blk(blk)
                for i in range(TPB):
                    t = blk * TPB + i
                    ps = psp.tile([P, K], mybir.dt.float32)
                    for f in range(FCH):
                        first = f == 0
                        last = f == FCH - 1
                        wh = xh_t[f][:, i * P:(i + 1) * P]
                        wl = xl_t[f][:, i * P:(i + 1) * P]
                        nc.tensor.matmul(ps[:, 0:NH], wh, ch_sb[f][:, 0:NH],
                                         start=first, stop=False)
                        nc.tensor.matmul(ps[:, NH:K], wh, ch_sb[f][:, NH:K],
                                         start=first, stop=False)
                        nc.tensor.matmul(ps[:, 0:NH], wh, cl_sb[f][:, 0:NH],
                                         start=False, stop=False)
                        nc.tensor.matmul(ps[:, NH:K], wh, cl_sb[f][:, NH:K],
                                         start=False, stop=False)
                        nc.tensor.matmul(ps[:, 0:NH], wl, ch_sb[f][:, 0:NH],
                                         start=False, stop=last)
                        nc.tensor.matmul(ps[:, NH:K], wl, ch_sb[f][:, NH:K],
                                         start=False, stop=last)
                    a_sb = work.tile([P, K], mybir.dt.float32, tag="a")
                    nc.vector.tensor_sub(a_sb, ps, c2b)
                    mx = work.tile([P, 8], mybir.dt.float32, tag="mx")
                    nc.vector.max(out=mx, in_=a_sb)
                    ix = work.tile([P, 8], mybir.dt.uint32, tag="ix")
                    nc.vector.max_index(ix, mx, a_sb)
                    nc.sync.dma_start(out[t], ix[:, 0:1])
    nc.finalize()
    return nc


def _split_hi_lo(a):
    """Split fp32 into two bf16 terms: a ~= hi + lo with ~2^-17 rel residue."""
    import ml_dtypes
    hi = a.astype(ml_dtypes.bfloat16)
    lo = (a - hi.astype(np.float32)).astype(ml_dtypes.bfloat16)
    return hi, lo


def _get_nc(bl):
    if bl not in _NC_CACHE:
        _NC_CACHE[bl] = _build(bl)
    return _NC_CACHE[bl]


def kernel(X, centroids):
    X = np.ascontiguousarray(np.asarray(X, dtype=np.float32))
    C = np.ascontiguousarray(np.asarray(centroids, dtype=np.float32))
    assert X.shape == (B, F) and C.shape == (K, F)

    xt = np.ascontiguousarray(X.T)
    ct = np.ascontiguousarray(C.T)
    xh_all, xl_all = _split_hi_lo(xt)
    ch_t, cl_t = _split_hi_lo(ct)

    nc = _get_nc(BL)
    in_maps = []
    for c in range(NCORES):
        sl = slice(c * BL, (c + 1) * BL)
        in_maps.append({
            "xh": np.ascontiguousarray(xh_all[:, sl]),
            "xl": np.ascontiguousarray(xl_all[:, sl]),
            "ch": ch_t,
            "cl": cl_t,
            "cc": C,
        })
    res = run_bass_kernel_spmd(nc, in_maps, core_ids=list(range(NCORES)))
    out = np.concatenate([r["out"].reshape(-1) for r in res.results])
    return out.astype(np.int32)



# revision 3
# speedup vs baseline: 2.7854x; 2.7854x over previous
"""KMeans assignment (vq_codebook) Trainium2 kernel.

argmin_k ||x_b - c_k||^2 for X[65536,1024], C[1024,1024], 8 NeuronCores,
data-parallel over the batch (8192 rows/core), centroids replicated.

Math: argmin_k d2 = argmax_k (X@C^T - ||c||^2/2); row term ||x||^2 dropped.
The cross term is computed with a SINGLE float32r matmul pass: the PE reads
4-byte fp32 operands and truncates them to FP22 (s1-e10-m11) internally, and
at moving-dim >= 256 fp32r streams at the full bf16 rate — 3x fewer PE
cycles than a hi/lo bf16 3-pass scheme.

The -||c||^2/2 bias (host fp64) is preloaded into each PSUM tile by the
otherwise-idle Activation engine (PE accumulates on top with start=False),
so the DVE only runs max + max_index per tile — both off the critical path
of the PE. The first 4 tiles instead subtract the bias on the DVE
(start=True matmuls) so the very first matmul is not gated on the bias
broadcast DMA.

FP22 truncation noise in d2 has std ~7e-3; rows whose top-2 score gap is
inside a 12-sigma margin (~0.3% of rows) are re-scored exactly on the host
(top-2 values come back for free alongside the argmax), making the result
exact regardless of the PE's truncation details.
"""
import numpy as np
import concourse.bacc as bacc
import concourse.mybir as mybir
from concourse.tile import TileContext
from concourse.bass_utils import run_bass_kernel_spmd

B, F, K = 65536, 1024, 1024
NCORES = 8
BL = B // NCORES          # rows per core
P = 128
FCH = F // P              # 8 contraction chunks
NH = 512                  # psum half (max fp32 moving operand / bank)
BBLK = 1024               # rows per X DMA block (4KB lines: full DMA bandwidth)
NBLK = BL // BBLK
TPB = BBLK // P           # b-tiles per block
N_PRESUB = 4              # leading tiles that bias on the DVE instead
MARGIN = 0.08             # host re-score threshold on the top-2 score gap
DT = mybir.dt.float32r

_NC_CACHE = {}


def _build(bl):
    nblk = bl // BBLK
    nb = bl // P
    nc = bacc.Bacc("TRN2", target_bir_lowering=False)
    xt = nc.dram_tensor("xt", [F, bl], DT, kind="ExternalInput")
    ct = nc.dram_tensor("ct", [F, K], DT, kind="ExternalInput")
    c2h = nc.dram_tensor("c2h", [K], mybir.dt.float32, kind="ExternalInput")
    out = nc.dram_tensor("out", [nb, P, 1], mybir.dt.uint32, kind="ExternalOutput")
    mxo = nc.dram_tensor("mxo", [nb, P, 2], mybir.dt.float32, kind="ExternalOutput")

    with TileContext(nc) as tc:
        with (
            tc.tile_pool(name="cres", bufs=1) as cres,
            tc.tile_pool(name="xp", bufs=2) as xp,
            tc.tile_pool(name="work", bufs=3) as work,
            tc.tile_pool(name="psp", bufs=4, space="PSUM") as psp,
        ):
            # X chunk loads stream on the Pool queue, C/c2/outputs on SP,
            # bias preload copies on Activation — three independent tracks.
            def load_blk(blk):
                b0 = blk * BBLK
                ts = []
                for f in range(FCH):
                    t = xp.tile([P, BBLK], DT, tag=f"x{f}")
                    nc.gpsimd.dma_start(t, xt[f * P:(f + 1) * P, b0:b0 + BBLK])
                    ts.append(t)
                return ts

            # negated, broadcast to all 128 partitions; first on SP so the
            # Activation copies can start filling PSUM banks early
            c2b = cres.tile([P, K], mybir.dt.float32)
            nc.sync.dma_start(c2b, c2h[None, :].to_broadcast([P, K]))

            # resident transposed centroid chunks; per-chunk tiles keep the
            # first matmuls gated only on the chunks they actually read
            ch_sb = []
            for f in range(FCH):
                t = cres.tile([P, K], DT, tag=f"c{f}")
                nc.sync.dma_start(t, ct[f * P:(f + 1) * P, :])
                ch_sb.append(t)

            blk0_tiles = load_blk(0)

            for blk in range(nblk):
                x_t = blk0_tiles if blk == 0 else load_blk(blk)
                for i in range(TPB):
                    t = blk * TPB + i
                    presub = t < N_PRESUB
                    ps = psp.tile([P, K], mybir.dt.float32)
                    if not presub:
                        # preload -||c||^2/2 into the PSUM tile; matmuls
                        # accumulate the cross term on top
                        nc.scalar.copy(ps, c2b)
                    for f in range(FCH):
                        first = (f == 0) and presub
                        last = f == FCH - 1
                        w = x_t[f][:, i * P:(i + 1) * P]
                        nc.tensor.matmul(ps[:, 0:NH], w, ch_sb[f][:, 0:NH],
                                         start=first, stop=last,
                                         skip_group_check=True)
                        nc.tensor.matmul(ps[:, NH:K], w, ch_sb[f][:, NH:K],
                                         start=first, stop=last,
                                         skip_group_check=True)
                    if presub:
                        a_sb = work.tile([P, K], mybir.dt.float32, tag="a")
                        nc.vector.tensor_add(a_sb, ps, c2b)
                        src = a_sb
                    else:
                        src = ps
                    mx = work.tile([P, 8], mybir.dt.float32, tag="mx")
                    nc.vector.max(out=mx, in_=src)
                    ix = work.tile([P, 8], mybir.dt.uint32, tag="ix")
                    nc.vector.max_index(ix, mx, src)
                    nc.sync.dma_start(out[t], ix[:, 0:1])
                    nc.sync.dma_start(mxo[t], mx[:, 0:2])
    nc.finalize()
    return nc


def _get_nc(bl):
    if bl not in _NC_CACHE:
        _NC_CACHE[bl] = _build(bl)
    return _NC_CACHE[bl]


def kernel(X, centroids):
    X = np.ascontiguousarray(np.asarray(X, dtype=np.float32))
    C = np.ascontiguousarray(np.asarray(centroids, dtype=np.float32))
    assert X.shape == (B, F) and C.shape == (K, F)

    xt = np.ascontiguousarray(X.T)
    ct = np.ascontiguousarray(C.T)
    c2 = np.einsum("kf,kf->k", C.astype(np.float64), C.astype(np.float64))
    nc2h = (-0.5 * c2).astype(np.float32)

    nc = _get_nc(BL)
    in_maps = []
    for c in range(NCORES):
        sl = slice(c * BL, (c + 1) * BL)
        in_maps.append({
            "xt": np.ascontiguousarray(xt[:, sl]),
            "ct": ct,
            "c2h": nc2h,
        })
    res = run_bass_kernel_spmd(nc, in_maps, core_ids=list(range(NCORES)))
    out = np.concatenate([r["out"].reshape(-1) for r in res.results])
    out = out.astype(np.int32)
    mx = np.concatenate([r["mxo"].reshape(-1, 2) for r in res.results])

    # exact host re-score of rows whose top-2 gap is inside the fp22 noise
    # margin (~0.3% of rows): kills all residual argmin flips
    gap = mx[:, 0] - mx[:, 1]
    risky = np.flatnonzero(gap < MARGIN)
    if risky.size:
        Xr = X[risky].astype(np.float64)
        d2 = (-2.0 * (Xr @ C.T.astype(np.float64))) + c2[None, :]
        out[risky] = np.argmin(d2, axis=1).astype(np.int32)
    return out


# revision 4
# speedup vs baseline: 2.8435x; 1.0209x over previous
"""KMeans assignment (vq_codebook) Trainium2 kernel.

argmin_k ||x_b - c_k||^2 for X[65536,1024], C[1024,1024], 8 NeuronCores,
data-parallel over the batch (8192 rows/core), centroids replicated.

Math: argmin_k d2 = argmax_k (X@C^T - ||c||^2/2); row term ||x||^2 dropped.
The cross term is computed with a SINGLE float32r matmul pass: the PE reads
4-byte fp32 operands and truncates them to FP22 (s1-e10-m11) internally, and
at moving-dim >= 256 fp32r streams at the full bf16 rate — 3x fewer PE
cycles than a hi/lo bf16 3-pass scheme.

The -||c||^2/2 bias (host fp64) is preloaded into each PSUM tile by the
otherwise-idle Activation engine (PE accumulates on top with start=False),
so the DVE only runs max + max_index per tile — both off the critical path
of the PE. The first 4 tiles instead subtract the bias on the DVE
(start=True matmuls) so the very first matmul is not gated on the bias
broadcast DMA.

FP22 truncation noise in d2 has std ~7e-3; rows whose top-2 score gap is
inside a 12-sigma margin (~0.3% of rows) are re-scored exactly on the host
(top-2 values come back for free alongside the argmax), making the result
exact regardless of the PE's truncation details.
"""
import numpy as np
import concourse.bacc as bacc
import concourse.mybir as mybir
from concourse.tile import TileContext
from concourse.bass_utils import run_bass_kernel_spmd

B, F, K = 65536, 1024, 1024
NCORES = 8
BL = B // NCORES          # rows per core
P = 128
FCH = F // P              # 8 contraction chunks
NH = 512                  # psum half (max fp32 moving operand / bank)
BBLK = 512                # rows per X DMA block (2KB lines; 16 blocks double-buffer)
NBLK = BL // BBLK
TPB = BBLK // P           # b-tiles per block
N_PRESUB = 4              # leading tiles that bias on the DVE instead
MARGIN = 0.08             # host re-score threshold on the top-2 score gap
DT = mybir.dt.float32r

_NC_CACHE = {}


def _build(bl):
    nblk = bl // BBLK
    nb = bl // P
    nc = bacc.Bacc("TRN2", target_bir_lowering=False)
    xt = nc.dram_tensor("xt", [F, bl], DT, kind="ExternalInput")
    ct = nc.dram_tensor("ct", [F, K], DT, kind="ExternalInput")
    c2h = nc.dram_tensor("c2h", [K], mybir.dt.float32, kind="ExternalInput")
    out = nc.dram_tensor("out", [nb, P, 1], mybir.dt.uint32, kind="ExternalOutput")
    mxo = nc.dram_tensor("mxo", [nb, P, 2], mybir.dt.float32, kind="ExternalOutput")

    with TileContext(nc) as tc:
        with (
            tc.tile_pool(name="cres", bufs=1) as cres,
            tc.tile_pool(name="xp", bufs=2) as xp,
            tc.tile_pool(name="work", bufs=3) as work,
            tc.tile_pool(name="psp", bufs=4, space="PSUM") as psp,
        ):
            # X chunk loads stream on the Pool queue, C/c2/outputs on SP,
            # bias preload copies on Activation — three independent tracks.
            def load_blk(blk):
                b0 = blk * BBLK
                ts = []
                for f in range(FCH):
                    t = xp.tile([P, BBLK], DT, tag=f"x{f}")
                    nc.gpsimd.dma_start(t, xt[f * P:(f + 1) * P, b0:b0 + BBLK])
                    ts.append(t)
                return ts

            # negated, broadcast to all 128 partitions; first on SP so the
            # Activation copies can start filling PSUM banks early
            c2b = cres.tile([P, K], mybir.dt.float32)
            nc.sync.dma_start(c2b, c2h[None, :].to_broadcast([P, K]))

            # resident transposed centroid chunks; per-chunk tiles keep the
            # first matmuls gated only on the chunks they actually read
            ch_sb = []
            for f in range(FCH):
                t = cres.tile([P, K], DT, tag=f"c{f}")
                nc.sync.dma_start(t, ct[f * P:(f + 1) * P, :])
                ch_sb.append(t)

            blk0_tiles = load_blk(0)

            for blk in range(nblk):
                x_t = blk0_tiles if blk == 0 else load_blk(blk)
                for i in range(TPB):
                    t = blk * TPB + i
                    presub = t < N_PRESUB
                    ps = psp.tile([P, K], mybir.dt.float32)
                    if not presub:
                        # preload -||c||^2/2 into the PSUM tile; matmuls
                        # accumulate the cross term on top
                        nc.scalar.copy(ps, c2b)
                    for f in range(FCH):
                        first = (f == 0) and presub
                        last = f == FCH - 1
                        w = x_t[f][:, i * P:(i + 1) * P]
                        nc.tensor.matmul(ps[:, 0:NH], w, ch_sb[f][:, 0:NH],
                                         start=first, stop=last,
                                         skip_group_check=True)
                        nc.tensor.matmul(ps[:, NH:K], w, ch_sb[f][:, NH:K],
                                         start=first, stop=last,
                                         skip_group_check=True)
                    if presub:
                        a_sb = work.tile([P, K], mybir.dt.float32, tag="a")
                        nc.vector.tensor_add(a_sb, ps, c2b)
                        src = a_sb
                    else:
                        src = ps
                    mx = work.tile([P, 8], mybir.dt.float32, tag="mx")
                    nc.vector.max(out=mx, in_=src)
                    ix = work.tile([P, 8], mybir.dt.uint32, tag="ix")
                    nc.vector.max_index(ix, mx, src)
                    nc.sync.dma_start(out[t], ix[:, 0:1])
                    nc.sync.dma_start(mxo[t], mx[:, 0:2])
    nc.finalize()
    return nc


def _get_nc(bl):
    if bl not in _NC_CACHE:
        _NC_CACHE[bl] = _build(bl)
    return _NC_CACHE[bl]


def kernel(X, centroids):
    X = np.ascontiguousarray(np.asarray(X, dtype=np.float32))
    C = np.ascontiguousarray(np.asarray(centroids, dtype=np.float32))
    assert X.shape == (B, F) and C.shape == (K, F)

    xt = np.ascontiguousarray(X.T)
    ct = np.ascontiguousarray(C.T)
    c2 = np.einsum("kf,kf->k", C.astype(np.float64), C.astype(np.float64))
    nc2h = (-0.5 * c2).astype(np.float32)

    nc = _get_nc(BL)
    in_maps = []
    for c in range(NCORES):
        sl = slice(c * BL, (c + 1) * BL)
        in_maps.append({
            "xt": np.ascontiguousarray(xt[:, sl]),
            "ct": ct,
            "c2h": nc2h,
        })
    res = run_bass_kernel_spmd(nc, in_maps, core_ids=list(range(NCORES)))
    out = np.concatenate([r["out"].reshape(-1) for r in res.results])
    out = out.astype(np.int32)
    mx = np.concatenate([r["mxo"].reshape(-1, 2) for r in res.results])

    # exact host re-score of rows whose top-2 gap is inside the fp22 noise
    # margin (~0.3% of rows): kills all residual argmin flips
    gap = mx[:, 0] - mx[:, 1]
    risky = np.flatnonzero(gap < MARGIN)
    if risky.size:
        Xr = X[risky].astype(np.float64)
        d2 = (-2.0 * (Xr @ C.T.astype(np.float64))) + c2[None, :]
        out[risky] = np.argmin(d2, axis=1).astype(np.int32)
    return out


# revision 5
# speedup vs baseline: 3.5304x; 1.2416x over previous
"""KMeans assignment (vq_codebook) Trainium2 kernel.

argmin_k ||x_b - c_k||^2 for X[65536,1024], C[1024,1024], 8 NeuronCores,
data-parallel over the batch (8192 rows/core), centroids replicated.

Math: argmin_k d2 = argmax_k (X@C^T - ||c||^2/2); row term ||x||^2 dropped.

The cross term runs entirely on the PE in fp8 (e4m3) DoubleRow perf mode at
0.5 cycles/row — 2x the bf16/fp32r rate. Operands are split into two e4m3
planes each (X = X0+X1, C = C0+C1) and the three dominant product terms
  X0@C0 + X0@C1 + X1@C0
are computed with DoubleRow packing two (weight,ifmap) plane pairs per
matmul: 24 plane-terms/tile -> 24 instructions x 512 cols x 0.5 cycles
= 6144 PE cycles/tile, vs 8192 for a single fp32r pass and 24576 for the
previous hi/lo bf16 3-pass scheme.

The -||c||^2/2 bias (host fp64) is preloaded into each PSUM tile by the
otherwise-idle Activation engine (matmuls accumulate on top, start=False),
so the DVE only runs max + max_index per tile. The first 4 tiles instead
bias on the DVE so the first matmul is not gated on the bias broadcast.
X streams as one contiguous 12KB-per-partition DMA per 512-row block
(a per-pass layout starves the PE on the DMA queue).

Accuracy: the dropped X1@C1 term and e4m3 quantization give the device
scores a d2 noise std of ~0.04, flipping ~257 of 65536 argmins. Every
device tile also ships its top-2 score values (free: DVE max already
computes them); the host exactly re-scores rows whose top-2 gap is inside
MARGIN=0.25 (~7.5% of rows; every observed flip on HW sits below gap
0.084, a 3x margin). Device computes 100% of the B*K scores and argmaxes;
the host re-check makes the result exact to fp64 for all flagged rows.
"""
import numpy as np
import ml_dtypes
import concourse.bacc as bacc
import concourse.mybir as mybir
from concourse.tile import TileContext
from concourse.bass_utils import run_bass_kernel_spmd

B, F, K = 65536, 1024, 1024
NCORES = 8
BL = B // NCORES          # rows per core
P = 128
FCH = F // P              # 8 feature chunks
NPASS = 12                # DoubleRow passes per tile-half (3 terms x 4 chunk-pairs)
NH = 512                  # psum half (max fp32 moving operand / bank)
BBLK = 512                # rows per X DMA block
NBLK = BL // BBLK
TPB = BBLK // P           # b-tiles per block
N_PRESUB = 4              # leading tiles that bias on the DVE instead
MARGIN = 0.25             # host re-score threshold on the top-2 score gap
E4 = ml_dtypes.float8_e4m3
DT8 = mybir.dt.float8e4

# pass 3j+t covers feature chunks (2j, 2j+1) with term t: (x_plane, c_plane)
TERMS = [(0, 0), (0, 1), (1, 0)]   # X0@C0, X0@C1, X1@C0

_NC_CACHE = {}


def _build(bl):
    nblk = bl // BBLK
    nb = bl // P
    nc = bacc.Bacc("TRN2", target_bir_lowering=False)
    xb = nc.dram_tensor("xb", [P, nblk, NPASS, 2, BBLK], DT8, kind="ExternalInput")
    cw = nc.dram_tensor("cw", [NPASS, P, 2, K], DT8, kind="ExternalInput")
    c2h = nc.dram_tensor("c2h", [K], mybir.dt.float32, kind="ExternalInput")
    out = nc.dram_tensor("out", [nb, P, 1], mybir.dt.uint32, kind="ExternalOutput")
    mxo = nc.dram_tensor("mxo", [nb, P, 2], mybir.dt.float32, kind="ExternalOutput")

    with TileContext(nc) as tc:
        with (
            tc.tile_pool(name="cres", bufs=1) as cres,
            tc.tile_pool(name="xp", bufs=2) as xp,
            tc.tile_pool(name="work", bufs=3) as work,
            tc.tile_pool(name="psp", bufs=4, space="PSUM") as psp,
        ):
            # X blocks on the Pool queue (one big contiguous DMA per block),
            # C/c2/argmax outputs on SP, bias copies + top-2 outputs on Act.
            def load_blk(blk):
                t = xp.tile([P, NPASS, 2, BBLK], DT8, tag="xb")
                nc.gpsimd.dma_start(t, xb[:, blk])
                return t

            cw_sb = []
            for p_ in range(NPASS):
                t = cres.tile([P, 2, K], DT8, tag=f"c{p_}")
                nc.sync.dma_start(t, cw[p_])
                cw_sb.append(t)
            c2b = cres.tile([P, K], mybir.dt.float32)
            nc.sync.dma_start(c2b, c2h[None, :].to_broadcast([P, K]))

            blk0 = load_blk(0)

            for blk in range(nblk):
                x_t = blk0 if blk == 0 else load_blk(blk)
                for i in range(TPB):
                    t = blk * TPB + i
                    presub = t < N_PRESUB
                    ps = psp.tile([P, K], mybir.dt.float32)
                    if not presub:
                        nc.scalar.copy(ps, c2b)
                    for p_ in range(NPASS):
                        first = (p_ == 0) and presub
                        last = p_ == NPASS - 1
                        w = x_t[:, p_, :, i * P:(i + 1) * P]
                        nc.tensor.matmul(ps[:, 0:NH], w, cw_sb[p_][:, :, 0:NH],
                                         start=first, stop=last,
                                         perf_mode=mybir.MatmulPerfMode.DoubleRow,
                                         skip_group_check=True)
                        nc.tensor.matmul(ps[:, NH:K], w, cw_sb[p_][:, :, NH:K],
                                         start=first, stop=last,
                                         perf_mode=mybir.MatmulPerfMode.DoubleRow,
                                         skip_group_check=True)
                    if presub:
                        a_sb = work.tile([P, K], mybir.dt.float32, tag="a")
                        nc.vector.tensor_add(a_sb, ps, c2b)
                        src = a_sb
                    else:
                        src = ps
                    mx = work.tile([P, 8], mybir.dt.float32, tag="mx")
                    nc.vector.max(out=mx, in_=src)
                    ix = work.tile([P, 8], mybir.dt.uint32, tag="ix")
                    nc.vector.max_index(ix, mx, src)
                    nc.sync.dma_start(out[t], ix[:, 0:1])
                    nc.scalar.dma_start(mxo[t], mx[:, 0:2])
    nc.finalize()
    return nc


def _get_nc(bl):
    if bl not in _NC_CACHE:
        _NC_CACHE[bl] = _build(bl)
    return _NC_CACHE[bl]


def _make_in_maps(X, C):
    """Host prep: e4m3 plane splits + interleaved DoubleRow layouts."""
    X0 = X.astype(E4)
    X1 = (X - X0.astype(np.float32)).astype(E4)
    C0 = C.astype(E4)
    C1 = (C - C0.astype(np.float32)).astype(E4)
    xsrcs = (np.ascontiguousarray(X0.T), np.ascontiguousarray(X1.T))
    csrcs = (np.ascontiguousarray(C0.T), np.ascontiguousarray(C1.T))

    cwt = np.empty((NPASS, P, 2, K), dtype=E4)
    for j in range(FCH // 2):
        for tix, (xsel, csel) in enumerate(TERMS):
            for i in range(2):
                f = 2 * j + i
                cwt[3 * j + tix, :, i, :] = csrcs[csel][f * P:(f + 1) * P, :]

    c2 = np.einsum("kf,kf->k", C.astype(np.float64), C.astype(np.float64))
    nc2h = (-0.5 * c2).astype(np.float32)

    in_maps = []
    for c in range(NCORES):
        b0 = c * BL
        xbc = np.empty((P, NBLK, NPASS, 2, BBLK), dtype=E4)
        for j in range(FCH // 2):
            for tix, (xsel, csel) in enumerate(TERMS):
                for i in range(2):
                    f = 2 * j + i
                    blkview = xsrcs[xsel][f * P:(f + 1) * P,
                                          b0:b0 + BL].reshape(P, NBLK, BBLK)
                    xbc[:, :, 3 * j + tix, i, :] = blkview
        in_maps.append({"xb": xbc, "cw": cwt, "c2h": nc2h})
    return in_maps, c2


def kernel(X, centroids):
    X = np.ascontiguousarray(np.asarray(X, dtype=np.float32))
    C = np.ascontiguousarray(np.asarray(centroids, dtype=np.float32))
    assert X.shape == (B, F) and C.shape == (K, F)

    in_maps, c2 = _make_in_maps(X, C)
    nc = _get_nc(BL)
    res = run_bass_kernel_spmd(nc, in_maps, core_ids=list(range(NCORES)))
    out = np.concatenate([r["out"].reshape(-1) for r in res.results]).astype(np.int32)
    mx = np.concatenate([r["mxo"].reshape(-1, 2) for r in res.results])

    # exact host re-score of rows whose device top-2 gap is inside the fp8
    # noise margin: every fp8-induced argmin flip sits well below MARGIN
    gap = mx[:, 0] - mx[:, 1]
    risky = np.flatnonzero(gap < MARGIN)
    if risky.size:
        Xr = X[risky].astype(np.float64)
        d2 = (-2.0 * (Xr @ C.T.astype(np.float64))) + c2[None, :]
        out[risky] = np.argmin(d2, axis=1).astype(np.int32)
    return out


# revision 6
# speedup vs baseline: 3.6417x; 1.0315x over previous
"""KMeans assignment (vq_codebook) Trainium2 kernel.

argmin_k ||x_b - c_k||^2 for X[65536,1024], C[1024,1024], 8 NeuronCores,
data-parallel over the batch (8192 rows/core), centroids replicated.

Math: argmin_k d2 = argmax_k (X@C^T - ||c||^2/2); row term ||x||^2 dropped.

The cross term runs entirely on the PE in fp8 (e4m3) DoubleRow perf mode at
0.5 cycles/row — 2x the bf16/fp32r rate. Operands are split into two e4m3
planes each (X = X0+X1, C = C0+C1) and the three dominant product terms
  X0@C0 + X0@C1 + X1@C0
are computed with DoubleRow packing two (weight,ifmap) plane pairs per
matmul: 24 plane-terms/tile -> 24 instructions x 512 cols x 0.5 cycles
= 6144 PE cycles/tile, vs 8192 for a single fp32r pass and 24576 for the
previous hi/lo bf16 3-pass scheme.

The -||c||^2/2 bias (host fp64) is preloaded into each PSUM tile by the
otherwise-idle Activation engine (matmuls accumulate on top, start=False),
so the DVE only runs max + max_index per tile. The first 4 tiles instead
bias on the DVE so the first matmul is not gated on the bias broadcast.
X streams as one contiguous 12KB-per-partition DMA per 512-row block
(a per-pass layout starves the PE on the DMA queue).

Accuracy: the dropped X1@C1 term and e4m3 quantization give the device
scores a d2 noise std of ~0.04, flipping ~257 of 65536 argmins. Every
device tile also ships its top-2 score values (free: DVE max already
computes them); the host exactly re-scores rows whose top-2 gap is inside
MARGIN=0.25 (~7.5% of rows; every observed flip on HW sits below gap
0.084, a 3x margin). Device computes 100% of the B*K scores and argmaxes;
the host re-check makes the result exact to fp64 for all flagged rows.
"""
import numpy as np
import ml_dtypes
import concourse.bacc as bacc
import concourse.mybir as mybir
from concourse.tile import TileContext
from concourse.bass_utils import run_bass_kernel_spmd

B, F, K = 65536, 1024, 1024
NCORES = 8
BL = B // NCORES          # rows per core
P = 128
FCH = F // P              # 8 feature chunks
NPASS = 12                # DoubleRow passes per tile-half (3 terms x 4 chunk-pairs)
NH = 512                  # psum half (max fp32 moving operand / bank)
BBLK = 512                # rows per X DMA block
NBLK = BL // BBLK
TPB = BBLK // P           # b-tiles per block
N_PRESUB = 4              # leading tiles that bias on the DVE instead
MARGIN = 0.25             # host re-score threshold on the top-2 score gap
E4 = ml_dtypes.float8_e4m3
DT8 = mybir.dt.float8e4

# pass 3j+t covers feature chunks (2j, 2j+1) with term t: (x_plane, c_plane)
TERMS = [(0, 0), (0, 1), (1, 0)]   # X0@C0, X0@C1, X1@C0

_NC_CACHE = {}


def _build(bl):
    nblk = bl // BBLK
    nb = bl // P
    nc = bacc.Bacc("TRN2", target_bir_lowering=False)
    xb = nc.dram_tensor("xb", [P, nblk, NPASS, 2, BBLK], DT8, kind="ExternalInput")
    cw = nc.dram_tensor("cw", [NPASS, P, 2, K], DT8, kind="ExternalInput")
    c2h = nc.dram_tensor("c2h", [K], mybir.dt.float32, kind="ExternalInput")
    out = nc.dram_tensor("out", [nb, P, 1], mybir.dt.uint32, kind="ExternalOutput")
    mxo = nc.dram_tensor("mxo", [nb, P, 2], mybir.dt.float32, kind="ExternalOutput")

    with TileContext(nc) as tc:
        with (
            tc.tile_pool(name="cres", bufs=1) as cres,
            tc.tile_pool(name="xp", bufs=2) as xp,
            tc.tile_pool(name="work", bufs=3) as work,
            tc.tile_pool(name="psp", bufs=4, space="PSUM") as psp,
        ):
            # X blocks on the Pool queue (one big contiguous DMA per block),
            # C/c2/outputs on SP, bias copy-in + score copy-out on Act.
            def load_blk(blk):
                t = xp.tile([P, NPASS, 2, BBLK], DT8, tag="xb")
                nc.gpsimd.dma_start(t, xb[:, blk])
                return t

            cw_sb = []
            for p_ in range(NPASS):
                t = cres.tile([P, 2, K], DT8, tag=f"c{p_}")
                nc.sync.dma_start(t, cw[p_])
                cw_sb.append(t)
            c2b = cres.tile([P, K], mybir.dt.float32)
            nc.sync.dma_start(c2b, c2h[None, :].to_broadcast([P, K]))

            # block 0 arrives as 3 chunks so pass-0 data lands early
            blk0 = xp.tile([P, NPASS, 2, BBLK], DT8, tag="xb")
            for s in range(3):
                st = s * (NPASS // 3)
                en = (s + 1) * (NPASS // 3)
                nc.gpsimd.dma_start(blk0[:, st:en], xb[:, 0, st:en])

            def mm(ps, x_t, i, p_, presub):
                first = (p_ == 0) and presub
                last = p_ == NPASS - 1
                w = x_t[:, p_, :, i * P:(i + 1) * P]
                nc.tensor.matmul(ps[:, 0:NH], w, cw_sb[p_][:, :, 0:NH],
                                 start=first, stop=last,
                                 perf_mode=mybir.MatmulPerfMode.DoubleRow,
                                 skip_group_check=True)
                nc.tensor.matmul(ps[:, NH:K], w, cw_sb[p_][:, :, NH:K],
                                 start=first, stop=last,
                                 perf_mode=mybir.MatmulPerfMode.DoubleRow,
                                 skip_group_check=True)

            def epilogue(ps, t, presub):
                # move scores to SBUF so the PSUM banks recycle ~2.3us
                # earlier and the DVE skips the PSUM access penalty; the
                # presub (startup) tiles fold the bias in on the way out
                a_sb = work.tile([P, K], mybir.dt.float32, tag="a")
                if presub:
                    nc.vector.tensor_add(a_sb, ps, c2b)
                else:
                    nc.scalar.copy(a_sb, ps)
                mx = work.tile([P, 8], mybir.dt.float32, tag="mx")
                nc.vector.max(out=mx, in_=a_sb)
                ix = work.tile([P, 8], mybir.dt.uint32, tag="ix")
                nc.vector.max_index(ix, mx, a_sb)
                nc.sync.dma_start(out[t], ix[:, 0:1])
                nc.sync.dma_start(mxo[t], mx[:, 0:2])

            # block 0 runs pass-major across its 4 tiles (4x214ns per pass
            # ~= the C-chunk DMA arrival pace, so the PE rides the C stream
            # instead of idling); these tiles bias on the DVE (start=True)
            pss = []
            for i in range(TPB):
                pstile = psp.tile([P, K], mybir.dt.float32, tag="ps")
                pss.append(pstile)
            for p_ in range(NPASS):
                for i in range(TPB):
                    mm(pss[i], blk0, i, p_, True)
            for i in range(TPB):
                epilogue(pss[i], i, True)

            for blk in range(1, nblk):
                x_t = load_blk(blk)
                for i in range(TPB):
                    t = blk * TPB + i
                    ps = psp.tile([P, K], mybir.dt.float32, tag="ps")
                    nc.scalar.copy(ps, c2b)
                    for p_ in range(NPASS):
                        mm(ps, x_t, i, p_, False)
                    epilogue(ps, t, False)
    nc.finalize()
    return nc


def _get_nc(bl):
    if bl not in _NC_CACHE:
        _NC_CACHE[bl] = _build(bl)
    return _NC_CACHE[bl]


def _make_in_maps(X, C):
    """Host prep: e4m3 plane splits + interleaved DoubleRow layouts."""
    X0 = X.astype(E4)
    X1 = (X - X0.astype(np.float32)).astype(E4)
    C0 = C.astype(E4)
    C1 = (C - C0.astype(np.float32)).astype(E4)
    xsrcs = (np.ascontiguousarray(X0.T), np.ascontiguousarray(X1.T))
    csrcs = (np.ascontiguousarray(C0.T), np.ascontiguousarray(C1.T))

    cwt = np.empty((NPASS, P, 2, K), dtype=E4)
    for j in range(FCH // 2):
        for tix, (xsel, csel) in enumerate(TERMS):
            for i in range(2):
                f = 2 * j + i
                cwt[3 * j + tix, :, i, :] = csrcs[csel][f * P:(f + 1) * P, :]

    c2 = np.einsum("kf,kf->k", C.astype(np.float64), C.astype(np.float64))
    nc2h = (-0.5 * c2).astype(np.float32)

    in_maps = []
    for c in range(NCORES):
        b0 = c * BL
        xbc = np.empty((P, NBLK, NPASS, 2, BBLK), dtype=E4)
        for j in range(FCH // 2):
            for tix, (xsel, csel) in enumerate(TERMS):
                for i in range(2):
                    f = 2 * j + i
                    blkview = xsrcs[xsel][f * P:(f + 1) * P,
                                          b0:b0 + BL].reshape(P, NBLK, BBLK)
                    xbc[:, :, 3 * j + tix, i, :] = blkview
        in_maps.append({"xb": xbc, "cw": cwt, "c2h": nc2h})
    return in_maps, c2


def kernel(X, centroids):
    X = np.ascontiguousarray(np.asarray(X, dtype=np.float32))
    C = np.ascontiguousarray(np.asarray(centroids, dtype=np.float32))
    assert X.shape == (B, F) and C.shape == (K, F)

    in_maps, c2 = _make_in_maps(X, C)
    nc = _get_nc(BL)
    res = run_bass_kernel_spmd(nc, in_maps, core_ids=list(range(NCORES)))
    out = np.concatenate([r["out"].reshape(-1) for r in res.results]).astype(np.int32)
    mx = np.concatenate([r["mxo"].reshape(-1, 2) for r in res.results])

    # exact host re-score of rows whose device top-2 gap is inside the fp8
    # noise margin: every fp8-induced argmin flip sits well below MARGIN
    gap = mx[:, 0] - mx[:, 1]
    risky = np.flatnonzero(gap < MARGIN)
    if risky.size:
        Xr = X[risky].astype(np.float64)
        d2 = (-2.0 * (Xr @ C.T.astype(np.float64))) + c2[None, :]
        out[risky] = np.argmin(d2, axis=1).astype(np.int32)
    return out


# revision 8
# speedup vs baseline: 3.6515x; 1.0027x over previous
"""KMeans assignment (vq_codebook) Trainium2 kernel.

argmin_k ||x_b - c_k||^2 for X[65536,1024], C[1024,1024], 8 NeuronCores,
data-parallel over the batch (8192 rows/core), centroids replicated.

Math: argmin_k d2 = argmax_k (X@C^T - ||c||^2/2); row term ||x||^2 dropped.

The cross term runs entirely on the PE in fp8 (e4m3) DoubleRow perf mode at
0.5 cycles/row — 2x the bf16/fp32r rate. Operands are split into two e4m3
planes each (X = X0+X1, C = C0+C1) and the three dominant product terms
  X0@C0 + X0@C1 + X1@C0
are computed with DoubleRow packing two (weight,ifmap) plane pairs per
matmul: 24 plane-terms/tile -> 24 instructions x 512 cols x 0.5 cycles
= 6144 PE cycles/tile, vs 8192 for a single fp32r pass and 24576 for the
previous hi/lo bf16 3-pass scheme.

The -||c||^2/2 bias (host fp64) is preloaded into each PSUM tile by the
otherwise-idle Activation engine (matmuls accumulate on top, start=False),
so the DVE only runs max + max_index per tile. The first 4 tiles instead
bias on the DVE so the first matmul is not gated on the bias broadcast.
X streams as one contiguous 12KB-per-partition DMA per 512-row block
(a per-pass layout starves the PE on the DMA queue).

Accuracy: the dropped X1@C1 term and e4m3 quantization give the device
scores a d2 noise std of ~0.04, flipping ~257 of 65536 argmins. Every
device tile also ships its top-2 score values (free: DVE max already
computes them); the host exactly re-scores rows whose top-2 gap is inside
MARGIN=0.25 (~7.5% of rows; every observed flip on HW sits below gap
0.084, a 3x margin). Device computes 100% of the B*K scores and argmaxes;
the host re-check makes the result exact to fp64 for all flagged rows.
"""
import numpy as np
import ml_dtypes
import concourse.bacc as bacc
import concourse.mybir as mybir
from concourse.tile import TileContext
from concourse.bass_utils import run_bass_kernel_spmd

B, F, K = 65536, 1024, 1024
NCORES = 8
BL = B // NCORES          # rows per core
P = 128
FCH = F // P              # 8 feature chunks
NPASS = 12                # DoubleRow passes per tile-half (3 terms x 4 chunk-pairs)
NH = 512                  # psum half (max fp32 moving operand / bank)
BBLK = 512                # rows per X DMA block
NBLK = BL // BBLK
TPB = BBLK // P           # b-tiles per block
N_PRESUB = 4              # leading tiles that bias on the DVE instead
MARGIN = 0.25             # host re-score threshold on the top-2 score gap
E4 = ml_dtypes.float8_e4m3
DT8 = mybir.dt.float8e4

# pass 3j+t covers feature chunks (2j, 2j+1) with term t: (x_plane, c_plane)
TERMS = [(0, 0), (0, 1), (1, 0)]   # X0@C0, X0@C1, X1@C0

_NC_CACHE = {}


def _build(bl):
    nblk = bl // BBLK
    nb = bl // P
    nc = bacc.Bacc("TRN2", target_bir_lowering=False)
    xb = nc.dram_tensor("xb", [P, nblk, NPASS, 2, BBLK], DT8, kind="ExternalInput")
    cw = nc.dram_tensor("cw", [NPASS, P, 2, K], DT8, kind="ExternalInput")
    c2h = nc.dram_tensor("c2h", [K], mybir.dt.float32, kind="ExternalInput")
    out = nc.dram_tensor("out", [nb, P, 1], mybir.dt.uint32, kind="ExternalOutput")
    mxo = nc.dram_tensor("mxo", [nb, P, 2], mybir.dt.float32, kind="ExternalOutput")

    ntiles = nblk * TPB
    with TileContext(nc) as tc:
        with (
            tc.tile_pool(name="cres", bufs=1) as cres,
            tc.tile_pool(name="xp", bufs=2) as xp,
            tc.tile_pool(name="work", bufs=6) as work,
            tc.tile_pool(name="psp", bufs=4, space="PSUM") as psp,
        ):
            # X blocks on the Pool queue (one big contiguous DMA per block),
            # C/c2/outputs on SP, bias copy-in + score copy-out on Act.
            def load_blk(blk):
                t = xp.tile([P, NPASS, 2, BBLK], DT8, tag="xb")
                nc.gpsimd.dma_start(t, xb[:, blk])
                return t

            cw_sb = []
            for p_ in range(NPASS):
                t = cres.tile([P, 2, K], DT8, tag=f"c{p_}")
                nc.sync.dma_start(t, cw[p_])
                cw_sb.append(t)
            c2b = cres.tile([P, K], mybir.dt.float32)
            nc.sync.dma_start(c2b, c2h[None, :].to_broadcast([P, K]))

            # block 0 arrives as 3 chunks so pass-0 data lands early
            blk0 = xp.tile([P, NPASS, 2, BBLK], DT8, tag="xb")
            for s in range(3):
                st = s * (NPASS // 3)
                en = (s + 1) * (NPASS // 3)
                nc.gpsimd.dma_start(blk0[:, st:en], xb[:, 0, st:en])

            def mm(ps, x_t, i, p_, presub):
                first = (p_ == 0) and presub
                last = p_ == NPASS - 1
                w = x_t[:, p_, :, i * P:(i + 1) * P]
                nc.tensor.matmul(ps[:, 0:NH], w, cw_sb[p_][:, :, 0:NH],
                                 start=first, stop=last,
                                 perf_mode=mybir.MatmulPerfMode.DoubleRow,
                                 skip_group_check=True)
                nc.tensor.matmul(ps[:, NH:K], w, cw_sb[p_][:, :, NH:K],
                                 start=first, stop=last,
                                 perf_mode=mybir.MatmulPerfMode.DoubleRow,
                                 skip_group_check=True)

            def finish(src, t):
                mx = work.tile([P, 8], mybir.dt.float32, tag="mx")
                nc.vector.max(out=mx, in_=src)
                ix = work.tile([P, 8], mybir.dt.uint32, tag="ix")
                nc.vector.max_index(ix, mx, src)
                nc.sync.dma_start(out[t], ix[:, 0:1])
                nc.sync.dma_start(mxo[t], mx[:, 0:2])

            # block 0 runs pass-major across its 4 tiles (4x214ns per pass
            # ~= the C-chunk DMA arrival pace, so the PE rides the C stream
            # instead of idling); these tiles bias on the DVE (start=True).
            # All 4 bias-adds are emitted BEFORE the max scans: the adds
            # release the PSUM banks at ~1.2us intervals instead of ~3.5us,
            # which keeps block 1's PE fed.
            pss = []
            for i in range(TPB):
                pstile = psp.tile([P, K], mybir.dt.float32, tag="ps")
                pss.append(pstile)
            for p_ in range(NPASS):
                for i in range(TPB):
                    mm(pss[i], blk0, i, p_, True)
            srcs = []
            for i in range(TPB):
                a_sb = work.tile([P, K], mybir.dt.float32, tag="a")
                nc.vector.tensor_add(a_sb, pss[i], c2b)
                srcs.append(a_sb)
            for i in range(TPB):
                finish(srcs[i], i)

            for blk in range(1, nblk):
                x_t = load_blk(blk)
                for i in range(TPB):
                    t = blk * TPB + i
                    ps = psp.tile([P, K], mybir.dt.float32, tag="ps")
                    nc.scalar.copy(ps, c2b)
                    for p_ in range(NPASS):
                        mm(ps, x_t, i, p_, False)
                    if t >= ntiles - 4:
                        # tail tiles: banks need no recycling, skip the
                        # copy-out hop to shorten the final serial chain
                        finish(ps, t)
                    else:
                        # move scores to SBUF so the PSUM banks recycle
                        # ~2.3us earlier and the DVE skips the PSUM access
                        # penalty
                        a_sb = work.tile([P, K], mybir.dt.float32, tag="a")
                        nc.scalar.copy(a_sb, ps)
                        finish(a_sb, t)
    nc.finalize()
    return nc


def _get_nc(bl):
    if bl not in _NC_CACHE:
        _NC_CACHE[bl] = _build(bl)
    return _NC_CACHE[bl]


def _make_in_maps(X, C):
    """Host prep: e4m3 plane splits + interleaved DoubleRow layouts."""
    X0 = X.astype(E4)
    X1 = (X - X0.astype(np.float32)).astype(E4)
    C0 = C.astype(E4)
    C1 = (C - C0.astype(np.float32)).astype(E4)
    xsrcs = (np.ascontiguousarray(X0.T), np.ascontiguousarray(X1.T))
    csrcs = (np.ascontiguousarray(C0.T), np.ascontiguousarray(C1.T))

    cwt = np.empty((NPASS, P, 2, K), dtype=E4)
    for j in range(FCH // 2):
        for tix, (xsel, csel) in enumerate(TERMS):
            for i in range(2):
                f = 2 * j + i
                cwt[3 * j + tix, :, i, :] = csrcs[csel][f * P:(f + 1) * P, :]

    c2 = np.einsum("kf,kf->k", C.astype(np.float64), C.astype(np.float64))
    nc2h = (-0.5 * c2).astype(np.float32)

    in_maps = []
    for c in range(NCORES):
        b0 = c * BL
        xbc = np.empty((P, NBLK, NPASS, 2, BBLK), dtype=E4)
        for j in range(FCH // 2):
            for tix, (xsel, csel) in enumerate(TERMS):
                for i in range(2):
                    f = 2 * j + i
                    blkview = xsrcs[xsel][f * P:(f + 1) * P,
                                          b0:b0 + BL].reshape(P, NBLK, BBLK)
                    xbc[:, :, 3 * j + tix, i, :] = blkview
        in_maps.append({"xb": xbc, "cw": cwt, "c2h": nc2h})
    return in_maps, c2


def kernel(X, centroids):
    X = np.ascontiguousarray(np.asarray(X, dtype=np.float32))
    C = np.ascontiguousarray(np.asarray(centroids, dtype=np.float32))
    assert X.shape == (B, F) and C.shape == (K, F)

    in_maps, c2 = _make_in_maps(X, C)
    nc = _get_nc(BL)
    res = run_bass_kernel_spmd(nc, in_maps, core_ids=list(range(NCORES)))
    out = np.concatenate([r["out"].reshape(-1) for r in res.results]).astype(np.int32)
    mx = np.concatenate([r["mxo"].reshape(-1, 2) for r in res.results])

    # exact host re-score of rows whose device top-2 gap is inside the fp8
    # noise margin: every fp8-induced argmin flip sits well below MARGIN
    gap = mx[:, 0] - mx[:, 1]
    risky = np.flatnonzero(gap < MARGIN)
    if risky.size:
        Xr = X[risky].astype(np.float64)
        d2 = (-2.0 * (Xr @ C.T.astype(np.float64))) + c2[None, :]
        out[risky] = np.argmin(d2, axis=1).astype(np.int32)
    return out


# revision 9
# speedup vs baseline: 3.6597x; 1.0023x over previous
"""KMeans assignment (vq_codebook) Trainium2 kernel.

argmin_k ||x_b - c_k||^2 for X[65536,1024], C[1024,1024], 8 NeuronCores,
data-parallel over the batch (8192 rows/core), centroids replicated.

Math: argmin_k d2 = argmax_k (X@C^T - ||c||^2/2); row term ||x||^2 dropped.

The cross term runs entirely on the PE in fp8 (e4m3) DoubleRow perf mode at
0.5 cycles/row — 2x the bf16/fp32r rate. Operands are split into two e4m3
planes each (X = X0+X1, C = C0+C1) and the three dominant product terms
  X0@C0 + X0@C1 + X1@C0
are computed with DoubleRow packing two (weight,ifmap) plane pairs per
matmul: 24 plane-terms/tile -> 24 instructions x 512 cols x 0.5 cycles
= 6144 PE cycles/tile, vs 8192 for a single fp32r pass and 24576 for the
original hi/lo bf16 3-pass scheme. The shared X0/C0 planes are stored once
and re-read via strided plane APs (33% less X DMA traffic).

The -||c||^2/2 bias (host fp64) is preloaded into each PSUM tile by the
otherwise-idle Activation engine (matmuls accumulate on top, start=False);
finished score tiles are copied PSUM->SBUF by the Activation engine so the
PSUM banks recycle ~2.3us earlier and the DVE (max + max_index per tile)
skips the PSUM access penalty — the DVE runs at ~91% of the PE pace, so
this slack is what keeps the PE from stalling. The last 4 tiles skip the
copy-out (no recycling pressure) to shorten the final serial chain.

Block 0 runs pass-major across its 4 PSUM-resident tiles (PE consumption
per pass ~= the C-chunk DMA arrival pace, so the PE rides the C stream
instead of idling) and biases on the DVE (start=True); its 4 bias-adds are
emitted before the max scans so the PSUM banks release at ~1.2us intervals
instead of ~3.5us.

Accuracy: the dropped X1@C1 term and e4m3 quantization give the device
scores a d2 noise std of ~0.04, flipping ~257 of 65536 argmins. Every
device tile also ships its top-2 score values (free: DVE max already
computes them); the host exactly re-scores rows whose top-2 gap is inside
MARGIN=0.25 (~7.5% of rows; every observed flip on HW sits below gap
0.084, a 3x margin). Device computes 100% of the B*K scores and argmaxes;
the host re-check makes the result exact to fp64 for all flagged rows.
"""
import numpy as np
import ml_dtypes
import concourse.bacc as bacc
import concourse.mybir as mybir
from concourse.tile import TileContext
from concourse.bass_utils import run_bass_kernel_spmd

B, F, K = 65536, 1024, 1024
NCORES = 8
BL = B // NCORES          # rows per core
P = 128
FCH = F // P              # 8 feature chunks
NJ = FCH // 2             # 4 chunk pairs
NPASS = 3 * NJ            # 12 DoubleRow passes per tile-half
NH = 512                  # psum half (max fp32 moving operand / bank)
BBLK = 512                # rows per X DMA block
NBLK = BL // BBLK
TPB = BBLK // P           # b-tiles per block
MARGIN = 0.25             # host re-score threshold on the top-2 score gap
E4 = ml_dtypes.float8_e4m3
DT8 = mybir.dt.float8e4

# pass 3j+t covers feature chunks (2j, 2j+1) with term t: (x_plane, c_plane)
TERMS = [(0, 0), (0, 1), (1, 0)]   # X0@C0, X0@C1, X1@C0

_NC_CACHE = {}


def _build(bl):
    nblk = bl // BBLK
    nb = bl // P
    ntiles = nblk * TPB
    nc = bacc.Bacc("TRN2", target_bir_lowering=False)
    # deduped planes: X [P, blk, j, i(chunk-in-pair), s(X0|X1), BBLK]
    xb = nc.dram_tensor("xb", [P, nblk, NJ, 2, 2, BBLK], DT8, kind="ExternalInput")
    # C [j, P, i, s(C0|C1), K]
    cw = nc.dram_tensor("cw", [NJ, P, 2, 2, K], DT8, kind="ExternalInput")
    c2h = nc.dram_tensor("c2h", [K], mybir.dt.float32, kind="ExternalInput")
    out = nc.dram_tensor("out", [nb, P, 1], mybir.dt.uint32, kind="ExternalOutput")
    mxo = nc.dram_tensor("mxo", [nb, P, 2], mybir.dt.float32, kind="ExternalOutput")

    with TileContext(nc) as tc:
        with (
            tc.tile_pool(name="cres", bufs=1) as cres,
            tc.tile_pool(name="xp", bufs=2) as xp,
            tc.tile_pool(name="work", bufs=6) as work,
            tc.tile_pool(name="psp", bufs=4, space="PSUM") as psp,
        ):
            # X blocks on the Pool queue (one contiguous DMA per block),
            # C/c2/outputs on SP, bias copy-in + score copy-out on Act.
            def load_blk(blk):
                t = xp.tile([P, NJ, 2, 2, BBLK], DT8, tag="xb")
                nc.gpsimd.dma_start(t, xb[:, blk])
                return t

            cw_sb = []
            for j in range(NJ):
                t = cres.tile([P, 2, 2, K], DT8, tag=f"c{j}")
                nc.sync.dma_start(t, cw[j])
                cw_sb.append(t)
            c2b = cres.tile([P, K], mybir.dt.float32)
            nc.sync.dma_start(c2b, c2h[None, :].to_broadcast([P, K]))

            # block 0 arrives as 2 chunks so early passes start sooner
            blk0 = xp.tile([P, NJ, 2, 2, BBLK], DT8, tag="xb")
            for s in range(2):
                nc.gpsimd.dma_start(blk0[:, 2 * s:2 * s + 2], xb[:, 0, 2 * s:2 * s + 2])

            def mm(ps, x_t, i, p_, presub):
                j, tix = divmod(p_, 3)
                xsel, csel = TERMS[tix]
                first = (p_ == 0) and presub
                last = p_ == NPASS - 1
                w = x_t[:, j, :, xsel, i * P:(i + 1) * P]
                r = cw_sb[j][:, :, csel, :]
                nc.tensor.matmul(ps[:, 0:NH], w, r[:, :, 0:NH],
                                 start=first, stop=last,
                                 perf_mode=mybir.MatmulPerfMode.DoubleRow,
                                 skip_group_check=True)
                nc.tensor.matmul(ps[:, NH:K], w, r[:, :, NH:K],
                                 start=first, stop=last,
                                 perf_mode=mybir.MatmulPerfMode.DoubleRow,
                                 skip_group_check=True)

            def finish(src, t):
                mx = work.tile([P, 8], mybir.dt.float32, tag="mx")
                nc.vector.max(out=mx, in_=src)
                ix = work.tile([P, 8], mybir.dt.uint32, tag="ix")
                nc.vector.max_index(ix, mx, src)
                nc.sync.dma_start(out[t], ix[:, 0:1])
                nc.sync.dma_start(mxo[t], mx[:, 0:2])

            # block 0: pass-major, bias on DVE, adds emitted before maxes
            pss = []
            for i in range(TPB):
                pstile = psp.tile([P, K], mybir.dt.float32, tag="ps")
                pss.append(pstile)
            for p_ in range(NPASS):
                for i in range(TPB):
                    mm(pss[i], blk0, i, p_, True)
            srcs = []
            for i in range(TPB):
                a_sb = work.tile([P, K], mybir.dt.float32, tag="a")
                nc.vector.tensor_add(a_sb, pss[i], c2b)
                srcs.append(a_sb)
            for i in range(TPB):
                finish(srcs[i], i)

            for blk in range(1, nblk):
                x_t = load_blk(blk)
                for i in range(TPB):
                    t = blk * TPB + i
                    ps = psp.tile([P, K], mybir.dt.float32, tag="ps")
                    nc.scalar.copy(ps, c2b)
                    for p_ in range(NPASS):
                        mm(ps, x_t, i, p_, False)
                    if t >= ntiles - 4:
                        # tail tiles: banks need no recycling, skip the
                        # copy-out hop to shorten the final serial chain
                        finish(ps, t)
                    else:
                        a_sb = work.tile([P, K], mybir.dt.float32, tag="a")
                        nc.scalar.copy(a_sb, ps)
                        finish(a_sb, t)
    nc.finalize()
    return nc


def _get_nc(bl):
    if bl not in _NC_CACHE:
        _NC_CACHE[bl] = _build(bl)
    return _NC_CACHE[bl]


def _make_in_maps(X, C):
    """Host prep: e4m3 plane splits + deduped DoubleRow layouts."""
    X0 = X.astype(E4)
    X1 = (X - X0.astype(np.float32)).astype(E4)
    C0 = C.astype(E4)
    C1 = (C - C0.astype(np.float32)).astype(E4)
    xsrcs = (np.ascontiguousarray(X0.T), np.ascontiguousarray(X1.T))
    csrcs = (np.ascontiguousarray(C0.T), np.ascontiguousarray(C1.T))

    cwt = np.empty((NJ, P, 2, 2, K), dtype=E4)
    for j in range(NJ):
        for i in range(2):
            f = 2 * j + i
            for s in range(2):
                cwt[j, :, i, s, :] = csrcs[s][f * P:(f + 1) * P, :]

    c2 = np.einsum("kf,kf->k", C.astype(np.float64), C.astype(np.float64))
    nc2h = (-0.5 * c2).astype(np.float32)

    in_maps = []
    for c in range(NCORES):
        b0 = c * BL
        xbc = np.empty((P, NBLK, NJ, 2, 2, BBLK), dtype=E4)
        for j in range(NJ):
            for i in range(2):
                f = 2 * j + i
                for s in range(2):
                    blkview = xsrcs[s][f * P:(f + 1) * P,
                                       b0:b0 + BL].reshape(P, NBLK, BBLK)
                    xbc[:, :, j, i, s, :] = blkview
        in_maps.append({"xb": xbc, "cw": cwt, "c2h": nc2h})
    return in_maps, c2


def kernel(X, centroids):
    X = np.ascontiguousarray(np.asarray(X, dtype=np.float32))
    C = np.ascontiguousarray(np.asarray(centroids, dtype=np.float32))
    assert X.shape == (B, F) and C.shape == (K, F)

    in_maps, c2 = _make_in_maps(X, C)
    nc = _get_nc(BL)
    res = run_bass_kernel_spmd(nc, in_maps, core_ids=list(range(NCORES)))
    out = np.concatenate([r["out"].reshape(-1) for r in res.results]).astype(np.int32)
    mx = np.concatenate([r["mxo"].reshape(-1, 2) for r in res.results])

    # exact host re-score of rows whose device top-2 gap is inside the fp8
    # noise margin: every fp8-induced argmin flip sits well below MARGIN
    gap = mx[:, 0] - mx[:, 1]
    risky = np.flatnonzero(gap < MARGIN)
    if risky.size:
        Xr = X[risky].astype(np.float64)
        d2 = (-2.0 * (Xr @ C.T.astype(np.float64))) + c2[None, :]
        out[risky] = np.argmin(d2, axis=1).astype(np.int32)
    return out


# revision 10
# speedup vs baseline: 3.7116x; 1.0142x over previous
"""KMeans assignment (vq_codebook) Trainium2 kernel.

argmin_k ||x_b - c_k||^2 for X[65536,1024], C[1024,1024], 8 NeuronCores,
data-parallel over the batch (8192 rows/core), centroids replicated.

Math: argmin_k d2 = argmax_k (X@C^T - ||c||^2/2); row term ||x||^2 dropped.

The cross term runs entirely on the PE in fp8 (e4m3) DoubleRow perf mode at
0.5 cycles/row — 2x the bf16/fp32r rate. Operands are split into two e4m3
planes each (X = X0+X1, C = C0+C1) and the three dominant product terms
  X0@C0 + X0@C1 + X1@C0
are computed with DoubleRow packing two (weight,ifmap) plane pairs per
matmul: 24 plane-terms/tile -> 24 instructions x 512 cols x 0.5 cycles
= 6144 PE cycles/tile, vs 8192 for a single fp32r pass and 24576 for the
original hi/lo bf16 3-pass scheme. The shared X0/C0 planes are stored once
and re-read via strided plane APs (33% less X DMA traffic).

The -||c||^2/2 bias (host fp64) is preloaded into each PSUM tile by the
otherwise-idle Activation engine (matmuls accumulate on top, start=False);
finished score tiles are copied PSUM->SBUF by the Activation engine so the
PSUM banks recycle ~2.3us earlier and the DVE (max + max_index per tile)
skips the PSUM access penalty — the DVE runs at ~91% of the PE pace, so
this slack is what keeps the PE from stalling. The last 4 tiles skip the
copy-out (no recycling pressure) to shorten the final serial chain.

Block 0 runs pass-major across its 4 PSUM-resident tiles (PE consumption
per pass ~= the C-chunk DMA arrival pace, so the PE rides the C stream
instead of idling) and biases on the DVE (start=True); its 4 bias-adds are
emitted before the max scans so the PSUM banks release at ~1.2us intervals
instead of ~3.5us.

Accuracy: the dropped X1@C1 term and e4m3 quantization give the device
scores a d2 noise std of ~0.04, flipping ~257 of 65536 argmins. Every
device tile also ships its top-2 score values (free: DVE max already
computes them); the host exactly re-scores rows whose top-2 gap is inside
MARGIN=0.25 (~7.5% of rows; every observed flip on HW sits below gap
0.084, a 3x margin). Device computes 100% of the B*K scores and argmaxes;
the host re-check makes the result exact to fp64 for all flagged rows.
"""
import numpy as np
import ml_dtypes
import concourse.bacc as bacc
import concourse.mybir as mybir
from concourse.tile import TileContext
from concourse.bass_utils import run_bass_kernel_spmd

B, F, K = 65536, 1024, 1024
NCORES = 8
BL = B // NCORES          # rows per core
P = 128
FCH = F // P              # 8 feature chunks
NJ = FCH // 2             # 4 chunk pairs
NPASS = 3 * NJ            # 12 DoubleRow passes per tile-half
NH = 512                  # psum half (max fp32 moving operand / bank)
BBLK = 512                # rows per X DMA block
NBLK = BL // BBLK
TPB = BBLK // P           # b-tiles per block
MARGIN = 0.25             # host re-score threshold on the top-2 score gap
E4 = ml_dtypes.float8_e4m3
DT8 = mybir.dt.float8e4

# pass 3j+t covers feature chunks (2j, 2j+1) with term t: (x_plane, c_plane)
TERMS = [(0, 0), (0, 1), (1, 0)]   # X0@C0, X0@C1, X1@C0

_NC_CACHE = {}


def _build(bl):
    nblk = bl // BBLK
    nb = bl // P
    ntiles = nblk * TPB
    nc = bacc.Bacc("TRN2", target_bir_lowering=False)
    # deduped planes: X [P, blk, j, i(chunk-in-pair), s(X0|X1), BBLK]
    xb = nc.dram_tensor("xb", [P, nblk, NJ, 2, 2, BBLK], DT8, kind="ExternalInput")
    # C [j, P, i, s(C0|C1), K]
    cw = nc.dram_tensor("cw", [NJ, P, 2, 2, K], DT8, kind="ExternalInput")
    c2h = nc.dram_tensor("c2h", [K], mybir.dt.float32, kind="ExternalInput")
    out = nc.dram_tensor("out", [nb, P, 1], mybir.dt.uint32, kind="ExternalOutput")
    mxo = nc.dram_tensor("mxo", [nb, P, 2], mybir.dt.float32, kind="ExternalOutput")

    with TileContext(nc) as tc:
        with (
            tc.tile_pool(name="cres", bufs=1) as cres,
            tc.tile_pool(name="xp", bufs=2) as xp,
            tc.tile_pool(name="work", bufs=6) as work,
            tc.tile_pool(name="psp", bufs=4, space="PSUM") as psp,
        ):
            # X blocks on the Pool queue (one contiguous DMA per block),
            # C/c2/outputs on SP, bias copy-in + score copy-out on Act.
            def load_blk(blk):
                t = xp.tile([P, NJ, 2, 2, BBLK], DT8, tag="xb")
                nc.gpsimd.dma_start(t, xb[:, blk])
                return t

            # the bias vector comes in as a tiny single-partition DMA and is
            # replicated on the Pool engine: a [P,K] broadcast DMA's write
            # burst starved the PE for ~290ns every matmul while it ran
            c2b = cres.tile([P, K], mybir.dt.float32)
            c2lin = cres.tile([1, K], mybir.dt.float32)
            nc.sync.dma_start(c2lin, c2h[None, :])
            cw_sb = []
            for j in range(NJ):
                t = cres.tile([P, 2, 2, K], DT8, tag=f"c{j}")
                nc.sync.dma_start(t, cw[j])
                cw_sb.append(t)
            nc.gpsimd.partition_broadcast(c2b, c2lin)

            # block 0 arrives as 2 chunks so early passes start sooner
            blk0 = xp.tile([P, NJ, 2, 2, BBLK], DT8, tag="xb")
            for s in range(2):
                nc.gpsimd.dma_start(blk0[:, 2 * s:2 * s + 2], xb[:, 0, 2 * s:2 * s + 2])

            def mm(ps, x_t, i, p_, presub):
                j, tix = divmod(p_, 3)
                xsel, csel = TERMS[tix]
                first = (p_ == 0) and presub
                last = p_ == NPASS - 1
                w = x_t[:, j, :, xsel, i * P:(i + 1) * P]
                r = cw_sb[j][:, :, csel, :]
                nc.tensor.matmul(ps[:, 0:NH], w, r[:, :, 0:NH],
                                 start=first, stop=last,
                                 perf_mode=mybir.MatmulPerfMode.DoubleRow,
                                 skip_group_check=True)
                nc.tensor.matmul(ps[:, NH:K], w, r[:, :, NH:K],
                                 start=first, stop=last,
                                 perf_mode=mybir.MatmulPerfMode.DoubleRow,
                                 skip_group_check=True)

            def finish(src, t):
                mx = work.tile([P, 8], mybir.dt.float32, tag="mx")
                nc.vector.max(out=mx, in_=src)
                ix = work.tile([P, 8], mybir.dt.uint32, tag="ix")
                nc.vector.max_index(ix, mx, src)
                nc.sync.dma_start(out[t], ix[:, 0:1])
                nc.sync.dma_start(mxo[t], mx[:, 0:2])

            # block 0: pass-major, bias on DVE, adds emitted before maxes
            pss = []
            for i in range(TPB):
                pstile = psp.tile([P, K], mybir.dt.float32, tag="ps")
                pss.append(pstile)
            for p_ in range(NPASS):
                for i in range(TPB):
                    mm(pss[i], blk0, i, p_, True)
            srcs = []
            for i in range(TPB):
                a_sb = work.tile([P, K], mybir.dt.float32, tag="a")
                nc.vector.tensor_add(a_sb, pss[i], c2b)
                srcs.append(a_sb)
            for i in range(TPB):
                finish(srcs[i], i)

            for blk in range(1, nblk):
                x_t = load_blk(blk)
                for i in range(TPB):
                    t = blk * TPB + i
                    ps = psp.tile([P, K], mybir.dt.float32, tag="ps")
                    nc.scalar.copy(ps, c2b)
                    for p_ in range(NPASS):
                        mm(ps, x_t, i, p_, False)
                    if t >= ntiles - 4:
                        # tail tiles: banks need no recycling, skip the
                        # copy-out hop to shorten the final serial chain
                        finish(ps, t)
                    else:
                        a_sb = work.tile([P, K], mybir.dt.float32, tag="a")
                        nc.scalar.copy(a_sb, ps)
                        finish(a_sb, t)
    nc.finalize()
    return nc


def _get_nc(bl):
    if bl not in _NC_CACHE:
        _NC_CACHE[bl] = _build(bl)
    return _NC_CACHE[bl]


def _make_in_maps(X, C):
    """Host prep: e4m3 plane splits + deduped DoubleRow layouts."""
    X0 = X.astype(E4)
    X1 = (X - X0.astype(np.float32)).astype(E4)
    C0 = C.astype(E4)
    C1 = (C - C0.astype(np.float32)).astype(E4)
    xsrcs = (np.ascontiguousarray(X0.T), np.ascontiguousarray(X1.T))
    csrcs = (np.ascontiguousarray(C0.T), np.ascontiguousarray(C1.T))

    cwt = np.empty((NJ, P, 2, 2, K), dtype=E4)
    for j in range(NJ):
        for i in range(2):
            f = 2 * j + i
            for s in range(2):
                cwt[j, :, i, s, :] = csrcs[s][f * P:(f + 1) * P, :]

    c2 = np.einsum("kf,kf->k", C.astype(np.float64), C.astype(np.float64))
    nc2h = (-0.5 * c2).astype(np.float32)

    in_maps = []
    for c in range(NCORES):
        b0 = c * BL
        xbc = np.empty((P, NBLK, NJ, 2, 2, BBLK), dtype=E4)
        for j in range(NJ):
            for i in range(2):
                f = 2 * j + i
                for s in range(2):
                    blkview = xsrcs[s][f * P:(f + 1) * P,
                                       b0:b0 + BL].reshape(P, NBLK, BBLK)
                    xbc[:, :, j, i, s, :] = blkview
        in_maps.append({"xb": xbc, "cw": cwt, "c2h": nc2h})
    return in_maps, c2


def kernel(X, centroids):
    X = np.ascontiguousarray(np.asarray(X, dtype=np.float32))
    C = np.ascontiguousarray(np.asarray(centroids, dtype=np.float32))
    assert X.shape == (B, F) and C.shape == (K, F)

    in_maps, c2 = _make_in_maps(X, C)
    nc = _get_nc(BL)
    res = run_bass_kernel_spmd(nc, in_maps, core_ids=list(range(NCORES)))
    out = np.concatenate([r["out"].reshape(-1) for r in res.results]).astype(np.int32)
    mx = np.concatenate([r["mxo"].reshape(-1, 2) for r in res.results])

    # exact host re-score of rows whose device top-2 gap is inside the fp8
    # noise margin: every fp8-induced argmin flip sits well below MARGIN
    gap = mx[:, 0] - mx[:, 1]
    risky = np.flatnonzero(gap < MARGIN)
    if risky.size:
        Xr = X[risky].astype(np.float64)
        d2 = (-2.0 * (Xr @ C.T.astype(np.float64))) + c2[None, :]
        out[risky] = np.argmin(d2, axis=1).astype(np.int32)
    return out


# revision 11
# speedup vs baseline: 3.7821x; 1.0190x over previous
"""KMeans assignment (vq_codebook) Trainium2 kernel.

argmin_k ||x_b - c_k||^2 for X[65536,1024], C[1024,1024], 8 NeuronCores,
data-parallel over the batch (8192 rows/core), centroids replicated.

Math: argmin_k d2 = argmax_k (X@C^T - ||c||^2/2); row term ||x||^2 dropped.

The cross term runs entirely on the PE in fp8 (e4m3) DoubleRow perf mode at
0.5 cycles/row — 2x the bf16/fp32r rate. Operands are split into two e4m3
planes each (X = X0+X1, C = C0+C1) and the three dominant product terms
  X0@C0 + X0@C1 + X1@C0
are computed with DoubleRow packing two (weight,ifmap) plane pairs per
matmul: 24 plane-terms/tile -> 24 instructions x 512 cols x 0.5 cycles
= 6144 PE cycles/tile, vs 8192 for a single fp32r pass and 24576 for the
original hi/lo bf16 3-pass scheme. The shared X0/C0 planes are stored once
and re-read via strided plane APs (33% less X DMA traffic).

The -||c||^2/2 bias (host fp64) is preloaded into each PSUM tile by the
otherwise-idle Activation engine (matmuls accumulate on top, start=False);
finished score tiles are copied PSUM->SBUF by the Activation engine so the
PSUM banks recycle ~2.3us earlier and the DVE (max + max_index per tile)
skips the PSUM access penalty — the DVE runs at ~91% of the PE pace, so
this slack is what keeps the PE from stalling. The last 4 tiles skip the
copy-out (no recycling pressure) to shorten the final serial chain.

Block 0 runs pass-major across its 4 PSUM-resident tiles (PE consumption
per pass ~= the C-chunk DMA arrival pace, so the PE rides the C stream
instead of idling) and biases on the DVE (start=True); its 4 bias-adds are
emitted before the max scans so the PSUM banks release at ~1.2us intervals
instead of ~3.5us.

Accuracy: the dropped X1@C1 term and e4m3 quantization give the device
scores a d2 noise std of ~0.04, flipping ~257 of 65536 argmins. Every
device tile also ships its top-2 score values (free: DVE max already
computes them); the host exactly re-scores rows whose top-2 gap is inside
MARGIN=0.25 (~7.5% of rows; every observed flip on HW sits below gap
0.084, a 3x margin). Device computes 100% of the B*K scores and argmaxes;
the host re-check makes the result exact to fp64 for all flagged rows.
"""
import numpy as np
import ml_dtypes
import concourse.bacc as bacc
import concourse.mybir as mybir
from concourse.tile import TileContext
from concourse.bass_utils import run_bass_kernel_spmd

B, F, K = 65536, 1024, 1024
NCORES = 8
BL = B // NCORES          # rows per core
P = 128
FCH = F // P              # 8 feature chunks
NJ = FCH // 2             # 4 chunk pairs
NPASS = 3 * NJ            # 12 DoubleRow passes per tile-half
NH = 512                  # psum half (max fp32 moving operand / bank)
BBLK = 512                # rows per X DMA block
NBLK = BL // BBLK
TPB = BBLK // P           # b-tiles per block
MARGIN = 0.25             # host re-score threshold on the top-2 score gap
E4 = ml_dtypes.float8_e4m3
DT8 = mybir.dt.float8e4

# pass 3j+t covers feature chunks (2j, 2j+1) with term t: (x_plane, c_plane)
TERMS = [(0, 0), (0, 1), (1, 0)]   # X0@C0, X0@C1, X1@C0

_NC_CACHE = {}


def _build(bl):
    nblk = bl // BBLK
    nb = bl // P
    ntiles = nblk * TPB
    nc = bacc.Bacc("TRN2", target_bir_lowering=False)
    # deduped planes: X [P, blk, j, i(chunk-in-pair), s(X0|X1), BBLK]
    xb = nc.dram_tensor("xb", [P, nblk, NJ, 2, 2, BBLK], DT8, kind="ExternalInput")
    # C [j, P, i, s(C0|C1), K]
    cw = nc.dram_tensor("cw", [NJ, P, 2, 2, K], DT8, kind="ExternalInput")
    c2h = nc.dram_tensor("c2h", [K], mybir.dt.float32, kind="ExternalInput")
    out = nc.dram_tensor("out", [nb, P, 1], mybir.dt.uint32, kind="ExternalOutput")
    mxo = nc.dram_tensor("mxo", [nb, P, 2], mybir.dt.float32, kind="ExternalOutput")

    with TileContext(nc) as tc:
        with (
            tc.tile_pool(name="cres", bufs=1) as cres,
            tc.tile_pool(name="xp", bufs=2) as xp,
            tc.tile_pool(name="work", bufs=6) as work,
            tc.tile_pool(name="psp", bufs=4, space="PSUM") as psp,
        ):
            # X blocks on the Pool queue (one contiguous DMA per block),
            # C/c2/outputs on SP, bias copy-in + score copy-out on Act.
            def load_blk(blk):
                t = xp.tile([P, NJ, 2, 2, BBLK], DT8, tag="xb")
                nc.gpsimd.dma_start(t, xb[:, blk])
                return t

            # the bias vector comes in as a tiny single-partition DMA and is
            # replicated on the Pool engine: a [P,K] broadcast DMA's write
            # burst starved the PE for ~290ns every matmul while it ran
            c2b = cres.tile([P, K], mybir.dt.float32)
            c2lin = cres.tile([1, K], mybir.dt.float32)
            nc.sync.dma_start(c2lin, c2h[None, :])
            cw_sb = []
            for j in range(NJ):
                t = cres.tile([P, 2, 2, K], DT8, tag=f"c{j}")
                nc.sync.dma_start(t, cw[j])
                cw_sb.append(t)
            nc.gpsimd.partition_broadcast(c2b, c2lin)

            # block 0 arrives as 2 chunks so early passes start sooner
            blk0 = xp.tile([P, NJ, 2, 2, BBLK], DT8, tag="xb")
            for s in range(2):
                nc.gpsimd.dma_start(blk0[:, 2 * s:2 * s + 2], xb[:, 0, 2 * s:2 * s + 2])

            def mm(ps, x_t, i, p_, presub):
                j, tix = divmod(p_, 3)
                xsel, csel = TERMS[tix]
                first = (p_ == 0) and presub
                last = p_ == NPASS - 1
                w = x_t[:, j, :, xsel, i * P:(i + 1) * P]
                r = cw_sb[j][:, :, csel, :]
                nc.tensor.matmul(ps[:, 0:NH], w, r[:, :, 0:NH],
                                 start=first, stop=last,
                                 perf_mode=mybir.MatmulPerfMode.DoubleRow,
                                 skip_group_check=True)
                nc.tensor.matmul(ps[:, NH:K], w, r[:, :, NH:K],
                                 start=first, stop=last,
                                 perf_mode=mybir.MatmulPerfMode.DoubleRow,
                                 skip_group_check=True)

            def finish(src, t):
                mx = work.tile([P, 8], mybir.dt.float32, tag="mx")
                nc.vector.max(out=mx, in_=src)
                ix = work.tile([P, 8], mybir.dt.uint32, tag="ix")
                nc.vector.max_index(ix, mx, src)
                nc.sync.dma_start(out[t], ix[:, 0:1])
                nc.sync.dma_start(mxo[t], mx[:, 0:2])

            def reg_tile(x_t, t, nocopy):
                ps = psp.tile([P, K], mybir.dt.float32, tag="ps")
                nc.scalar.copy(ps, c2b)
                for p_ in range(NPASS):
                    mm(ps, x_t, t % TPB, p_, False)
                if nocopy:
                    # tail tiles: banks need no recycling, skip the
                    # copy-out hop to shorten the final serial chain
                    finish(ps, t)
                else:
                    a_sb = work.tile([P, K], mybir.dt.float32, tag="a")
                    nc.scalar.copy(a_sb, ps)
                    finish(a_sb, t)

            # block 0: first 3 tiles pass-major, bias on DVE, adds emitted
            # before maxes. Tile 3 takes the regular Act path (its PSUM bank
            # is never touched by the pass-major group), which warms the Act
            # copy-in/copy-out pipeline before block 1 needs it.
            PM = 3
            pss = []
            for i in range(PM):
                pstile = psp.tile([P, K], mybir.dt.float32, tag="ps")
                pss.append(pstile)
            for p_ in range(NPASS):
                for i in range(PM):
                    mm(pss[i], blk0, i, p_, True)
            srcs = []
            for i in range(PM):
                a_sb = work.tile([P, K], mybir.dt.float32, tag="a")
                nc.vector.tensor_add(a_sb, pss[i], c2b)
                srcs.append(a_sb)
            for i in range(PM):
                finish(srcs[i], i)
            for i in range(PM, TPB):
                reg_tile(blk0, i, False)

            for blk in range(1, nblk):
                x_t = load_blk(blk)
                for i in range(TPB):
                    t = blk * TPB + i
                    reg_tile(x_t, t, t >= ntiles - 4)
    nc.finalize()
    return nc


def _get_nc(bl):
    if bl not in _NC_CACHE:
        _NC_CACHE[bl] = _build(bl)
    return _NC_CACHE[bl]


def _make_in_maps(X, C):
    """Host prep: e4m3 plane splits + deduped DoubleRow layouts."""
    X0 = X.astype(E4)
    X1 = (X - X0.astype(np.float32)).astype(E4)
    C0 = C.astype(E4)
    C1 = (C - C0.astype(np.float32)).astype(E4)
    xsrcs = (np.ascontiguousarray(X0.T), np.ascontiguousarray(X1.T))
    csrcs = (np.ascontiguousarray(C0.T), np.ascontiguousarray(C1.T))

    cwt = np.empty((NJ, P, 2, 2, K), dtype=E4)
    for j in range(NJ):
        for i in range(2):
            f = 2 * j + i
            for s in range(2):
                cwt[j, :, i, s, :] = csrcs[s][f * P:(f + 1) * P, :]

    c2 = np.einsum("kf,kf->k", C.astype(np.float64), C.astype(np.float64))
    nc2h = (-0.5 * c2).astype(np.float32)

    in_maps = []
    for c in range(NCORES):
        b0 = c * BL
        xbc = np.empty((P, NBLK, NJ, 2, 2, BBLK), dtype=E4)
        for j in range(NJ):
            for i in range(2):
                f = 2 * j + i
                for s in range(2):
                    blkview = xsrcs[s][f * P:(f + 1) * P,
                                       b0:b0 + BL].reshape(P, NBLK, BBLK)
                    xbc[:, :, j, i, s, :] = blkview
        in_maps.append({"xb": xbc, "cw": cwt, "c2h": nc2h})
    return in_maps, c2


def kernel(X, centroids):
    X = np.ascontiguousarray(np.asarray(X, dtype=np.float32))
    C = np.ascontiguousarray(np.asarray(centroids, dtype=np.float32))
    assert X.shape == (B, F) and C.shape == (K, F)

    in_maps, c2 = _make_in_maps(X, C)
    nc = _get_nc(BL)
    res = run_bass_kernel_spmd(nc, in_maps, core_ids=list(range(NCORES)))
    out = np.concatenate([r["out"].reshape(-1) for r in res.results]).astype(np.int32)
    mx = np.concatenate([r["mxo"].reshape(-1, 2) for r in res.results])

    # exact host re-score of rows whose device top-2 gap is inside the fp8
    # noise margin: every fp8-induced argmin flip sits well below MARGIN
    gap = mx[:, 0] - mx[:, 1]
    risky = np.flatnonzero(gap < MARGIN)
    if risky.size:
        Xr = X[risky].astype(np.float64)
        d2 = (-2.0 * (Xr @ C.T.astype(np.float64))) + c2[None, :]
        out[risky] = np.argmin(d2, axis=1).astype(np.int32)
    return out
